# revision 29
# baseline (speedup 1.0000x reference)
"""MoE routing kernel for Trainium2 (8 NeuronCores, batch-parallel).

Problem: nn_MoE_47278999994656.
  x [8, 256, 80, 80] f32 + gate Linear(256->5) + 5 experts
  (residual conv1x1 on each 128-ch half, gated by a sigmoid transform),
  top-1 masked-softmax gate => weights are EXACTLY one-hot, so
  out[b] = expert_{argmax_e logits[b,e]}(x[b]).

Core i computes batch item i. Per core, three phases:
  A) x is host-cast to bf16 and streamed on the two HWDGE queues
     (sync: half0, scalar: half1) in 4 pieces each; the gate runs as
     PSUM-accumulated [5,512] matmuls per piece, with junk matmuls
     interleaved to keep the PE p-state ramped.
  B) argmax is computed as a register (mask -> iota+min -> int32 ->
     values_load); the selected expert's weights are then fetched from
     DRAM with ONE dynamic-index DMA per region (no 5-expert masked
     sum on the vector engines).
  C) expert pipeline, stage-skewed (D(i), H(i-1), A(i-2)) with
     elementwise work spread over Vector/Scalar/GpSimd; output is
     written bf16 in [128,1024] blocks alternating both HWDGE queues.
"""

import numpy as np

import concourse.bacc as bacc_mod
import concourse.bass as bass
import concourse.mybir as mybir
import concourse.tile as tile
from concourse.bass import ds
from concourse.bass_utils import run_bass_kernel_spmd

B, C, H, W = 8, 256, 80, 80
HW = H * W          # 6400
HALF = 128
QUARTER = 64
E = 5
NCORES = 8

# expert-layer chunks: 12 x 512 + 1 x 256 (psum bank holds 512 f32)
CHUNKS = [(i * 512, 512) for i in range(12)] + [(6144, 256)]
# input DMA: the two HWDGE queues run ~190 GB/s each with ~1.5k-col
# pieces; SWDGE starts ~10us late (engine preamble) so it only carries
# the tiny gate weights. sync: h0, scalar: h1, 4 pieces each.
HP = [(0, 1472), (1472, 1472), (2944, 1472), (4416, 1984)]
# (half, off, n) in arrival order for the gate matmuls
ARRIVAL = []
for _o, _n in HP:
    ARRIVAL.append((0, _o, _n))
    ARRIVAL.append((1, _o, _n))

# U_all free-dim layout (per expert, partition dim = 128):
#   [0:128)    (I + Wrgb)^T        [c, o]
#   [128:256)  (I + Wtir)^T        [c, o]
#   [256:320)  Wt1^T               [o, m]   (m = 64)
#   [320:448)  Wt2 replicated      [m, :]   rows 0:64 and 64:128 both = rep
UF = 448
U_WT1 = 256
U_WT2 = 320

F32 = mybir.dt.float32
BF16 = mybir.dt.bfloat16
I32 = mybir.dt.int32


def build_nc() -> bass.Bass:
    nc = bacc_mod.Bacc(num_devices=NCORES)

    x_d = nc.dram_tensor("x", [C, HW], BF16, kind="ExternalInput")
    # u[e] stored flat as [8, 7168] so the runtime-indexed fetch needs only
    # 8 big descriptors (dynamic-offset DMA pays ~125ns/descriptor in ucode
    # descgen); a static SBUF->SBUF DMA then reshapes to [128, 448].
    u_d = nc.dram_tensor("u", [E, 8, 16 * UF], BF16, kind="ExternalInput")
    bias_d = nc.dram_tensor("bias", [E, 1, HALF * 4], F32, kind="ExternalInput")
    wg_d = nc.dram_tensor("wg", [HALF, 2, E], BF16, kind="ExternalInput")
    bg_d = nc.dram_tensor("bg", [1, E], F32, kind="ExternalInput")
    iota_d = nc.dram_tensor("iota", [1, E], F32, kind="ExternalInput")
    out_d = nc.dram_tensor("out", [HALF, HW], BF16, kind="ExternalOutput")

    with tile.TileContext(nc) as tc:
        with (
            tc.tile_pool(name="big", bufs=1) as big,
            tc.tile_pool(name="const", bufs=1) as const,
            tc.tile_pool(name="small", bufs=1) as small,
            tc.tile_pool(name="hpool", bufs=6) as hpool,
            tc.tile_pool(name="ppool", bufs=6) as ppool,
            tc.tile_pool(name="dps_p", bufs=4, space="PSUM") as dps_p,
            tc.tile_pool(name="hps_p", bufs=2, space="PSUM") as hps_p,
            tc.tile_pool(name="aps_p", bufs=1, space="PSUM") as aps_p,
        ):
            # ---- persistent SBUF ----
            xb = big.tile([HALF, 2, HW], BF16)       # 25.6 KB/part
            dsb = big.tile([HALF, 2, HW], BF16)      # 25.6 KB/part
            osb = big.tile([HALF, HW], BF16)         # 12.8 KB/part
            usel = const.tile([HALF, UF], BF16)
            ubnc = const.tile([8, 16, UF], BF16)
            bsel = const.tile([HALF, 4], F32)
            bbnc = const.tile([1, HALF, 4], F32)
            wg = const.tile([HALF, 2, E], BF16)
            bgx = const.tile([1, E], F32)
            iot = const.tile([1, E], F32)

            # ---- phase A: x on both HWDGE queues, tiny weights on SWDGE ----
            nc.gpsimd.dma_start(out=wg[:], in_=wg_d[:])
            nc.gpsimd.dma_start(out=bgx[:], in_=bg_d[:])
            nc.gpsimd.dma_start(out=iot[:], in_=iota_d[:])
            # dummy activations so the act tables load during the DMA phase,
            # not at the first real activation inside the pipeline
            awarm = small.tile([1, 8], F32)
            nc.vector.memset(awarm, 0.0)
            nc.scalar.activation(
                out=awarm[0:1, 0:4], in_=awarm[0:1, 0:4],
                func=mybir.ActivationFunctionType.Identity,
            )
            nc.scalar.activation(
                out=awarm[0:1, 4:8], in_=awarm[0:1, 4:8],
                func=mybir.ActivationFunctionType.Sigmoid,
            )
            for o, n in HP:
                nc.sync.dma_start(
                    out=xb[:, 0, o : o + n], in_=x_d[0:HALF, o : o + n]
                )
                nc.scalar.dma_start(
                    out=xb[:, 1, o : o + n], in_=x_d[HALF:C, o : o + n]
                )

            # gate: yg[5, 512] += WgT_half^T @ xb sub-pieces (PSUM
            # accumulate), in arrival order. Junk matmuls keep the PE
            # continuously busy so its p-state ramps toward full clock
            # by the time the expert pipeline starts.
            yg = dps_p.tile([E, 512], F32, tag="dps", name="yg")
            ones1 = small.tile([1, 1], F32)
            nc.vector.memset(ones1, 1.0)

            def emit_junk(k, cols=512):
                for _ in range(k):
                    jt = dps_p.tile([E, 512], F32, tag="dps", name="junk")
                    nc.tensor.matmul(
                        jt[:, 0:cols], lhsT=wg[:, 0, :], rhs=xb[:, 0, 0:cols]
                    )

            # warm the PE before the first x piece lands (tiny wg junk)
            for _ in range(8):
                jt = dps_p.tile([E, 512], F32, tag="dps", name="jwarm")
                nc.tensor.matmul(jt[:, 0:E], lhsT=wg[:, 0, :],
                                 rhs=wg[:, 0, :])

            subs = []
            piece_of = []
            for pi, (h, o, n) in enumerate(ARRIVAL):
                done = 0
                while done < n:
                    m = min(512, n - done)
                    subs.append((h, o + done, m))
                    piece_of.append(pi)
                    done += m
            for k, (h, o, n) in enumerate(subs):
                nc.tensor.matmul(
                    yg[:, 0:n],
                    lhsT=wg[:, h, :],
                    rhs=xb[:, h, o : o + n],
                    start=(k == 0),
                    stop=False,
                )

            # fold the gate bias into column 0 of yg (saves a vector op)
            nc.tensor.matmul(
                yg[:, 0:1], lhsT=bgx[0:1, :], rhs=ones1[0:1, 0:1],
                start=False, stop=True,
            )

            # ---- phase B: argmax -> register -> dynamic weight fetch ----
            t32a = small.tile([32, 32], F32)
            t32b = small.tile([32, 32], F32)
            nc.vector.memset(t32a, 0.0)
            nc.vector.reduce_sum(t32a[0:E, 0:1], yg, axis=mybir.AxisListType.X)
            nc.vector.transpose(t32b, t32a)
            lmax = small.tile([1, 1], F32)
            nc.vector.reduce_max(lmax, t32b[0:1, 0:E], axis=mybir.AxisListType.X)
            # selrow[e] = is_eq(l[e], max) * (e - 200): flagged -> e-200 (<0)
            # reduce_min then picks the FIRST flagged index (ref tie-break).
            selrow = small.tile([1, E], F32)
            nc.vector.scalar_tensor_tensor(
                out=selrow, in0=t32b[0:1, 0:E], scalar=lmax[0:1, 0:1],
                in1=iot, op0=mybir.AluOpType.is_equal,
                op1=mybir.AluOpType.mult,
            )
            selv = small.tile([1, 1], F32)
            nc.vector.tensor_reduce(
                selv, selrow, axis=mybir.AxisListType.X, op=mybir.AluOpType.min
            )
            selint = small.tile([1, 1], I32)
            nc.vector.tensor_scalar(
                out=selint, in0=selv, scalar1=200.0, scalar2=None,
                op0=mybir.AluOpType.add,
            )
            sel = nc.values_load(
                selint[0:1, 0:1],
                engines=[mybir.EngineType.Pool, mybir.EngineType.SP,
                         mybir.EngineType.Activation],
                min_val=0, max_val=E - 1,
                skip_runtime_bounds_check=True,
            )
            # keep PE hot across the select window
            emit_junk(4)
            # selected expert's weights: dynamic-index fetch of the flat
            # block (few big descriptors), then static SBUF->SBUF reshapes
            # into lhsT layout. D columns land first so D(0) starts asap.
            nc.scalar.dma_start(out=bbnc[:], in_=bias_d[ds(sel, 1)])
            nc.sync.dma_start(out=ubnc[:], in_=u_d[ds(sel, 1)])
            nc.sync.dma_start(out=usel[:, 0:256], in_=ubnc[:, :, 0:256])
            nc.scalar.dma_start(out=bsel[:], in_=bbnc[0:1, :, :])
            nc.scalar.dma_start(out=usel[:, 256:UF], in_=ubnc[:, :, 256:UF])

            # ---- phase C: stage-skewed pipeline. PE iteration i issues
            # D(i), H(i-1), A(i-2). ----
            hsbl = [None] * len(CHUNKS)
            ssbl = [None] * len(CHUNKS)
            for i in range(len(CHUNKS) + 2):
                if i < len(CHUNKS):
                    off, n = CHUNKS[i]
                    dr = dps_p.tile([HALF, 512], F32, tag="dps", name="dr")
                    nc.tensor.matmul(
                        dr[:, 0:n], lhsT=usel[:, 0:HALF],
                        rhs=xb[:, 0, off : off + n]
                    )
                    dt = dps_p.tile([HALF, 512], F32, tag="dps", name="dt")
                    nc.tensor.matmul(
                        dt[:, 0:n], lhsT=usel[:, HALF : 2 * HALF],
                        rhs=xb[:, 1, off : off + n],
                    )
                    nc.vector.tensor_scalar_add(
                        dsb[:, 0, off : off + n], dr[:, 0:n], bsel[:, 0:1]
                    )
                    nc.scalar.activation(
                        out=dsb[:, 1, off : off + n], in_=dt[:, 0:n],
                        func=mybir.ActivationFunctionType.Identity,
                        bias=bsel[:, 1:2],
                    )
                if 0 <= i - 1 < len(CHUNKS):
                    ci = i - 1
                    off, n = CHUNKS[ci]
                    hps = hps_p.tile([HALF, 512], F32, tag="hps", name="hps")
                    nc.tensor.matmul(
                        hps[0:QUARTER, 0:n],
                        lhsT=usel[:, U_WT1 : U_WT1 + QUARTER],
                        rhs=dsb[:, 0, off : off + n],
                    )
                    nc.tensor.matmul(
                        hps[QUARTER:HALF, 0:n],
                        lhsT=usel[:, U_WT1 : U_WT1 + QUARTER],
                        rhs=dsb[:, 1, off : off + n],
                        tile_position=(0, QUARTER),
                    )
                    hsbl[ci] = hpool.tile([HALF, 512], BF16, tag="hsb", name="hsb")
                    if ci % 2 == 0:
                        nc.vector.tensor_scalar(
                            out=hsbl[ci][:, 0:n], in0=hps[:, 0:n],
                            scalar1=bsel[:, 2:3], scalar2=0.0,
                            op0=mybir.AluOpType.add, op1=mybir.AluOpType.max,
                        )
                    else:
                        nc.scalar.activation(
                            out=hsbl[ci][:, 0:n], in_=hps[:, 0:n],
                            func=mybir.ActivationFunctionType.Relu,
                            bias=bsel[:, 2:3],
                        )
                if 0 <= i - 2 < len(CHUNKS):
                    ci = i - 2
                    off, n = CHUNKS[ci]
                    aps = aps_p.tile([HALF, 2, 512], F32, tag="aps", name="aps")
                    nc.tensor.matmul(
                        aps[:, 0, 0:n],
                        lhsT=usel[0:QUARTER, U_WT2 : U_WT2 + HALF],
                        rhs=hsbl[ci][0:QUARTER, 0:n],
                        tile_position=(0, 0),
                    )
                    nc.tensor.matmul(
                        aps[:, 1, 0:n],
                        lhsT=usel[QUARTER:HALF, U_WT2 : U_WT2 + HALF],
                        rhs=hsbl[ci][QUARTER:HALF, 0:n],
                        tile_position=(QUARTER, 0),
                    )
                    ssbl[ci] = ppool.tile([HALF, 2, 512], BF16, tag="ssb", name="ssb")
                    nc.scalar.activation(
                        out=ssbl[ci][:, :, 0:n], in_=aps[:, :, 0:n],
                        func=mybir.ActivationFunctionType.Sigmoid,
                        bias=bsel[:, 3:4],
                    )
                    prt = ppool.tile([HALF, 2, 512], BF16, tag="prt", name="prt")
                    nc.vector.tensor_mul(
                        prt[:, :, 0:n], dsb[:, :, off : off + n],
                        ssbl[ci][:, :, 0:n]
                    )
                    nc.gpsimd.tensor_add(
                        osb[:, off : off + n], prt[:, 0, 0:n], prt[:, 1, 0:n]
                    )
                    if ci % 2 == 1:
                        bo = CHUNKS[ci - 1][0]
                        bn = off + n - bo
                        eng = nc.sync if (ci // 2) % 2 == 0 else nc.scalar
                        eng.dma_start(
                            out=out_d[:, bo : bo + bn], in_=osb[:, bo : bo + bn]
                        )
                    elif ci == len(CHUNKS) - 1:
                        nc.sync.dma_start(
                            out=out_d[:, off : off + n], in_=osb[:, off : off + n]
                        )

    nc.compile()
    return nc


def _pack_inputs(x, Wg, bg, Wrgb, brgb, Wtir, btir, Wt1, bt1, Wt2, bt2):
    import ml_dtypes
    eye = np.eye(HALF, dtype=np.float32)
    u = np.zeros((E, HALF, UF), dtype=np.float32)
    for e in range(E):
        u[e, :, 0:HALF] = Wrgb[e].T + eye
        u[e, :, HALF : 2 * HALF] = Wtir[e].T + eye
        u[e, :, U_WT1 : U_WT1 + QUARTER] = Wt1[e].T
        u[e, :, U_WT2 : U_WT2 + HALF] = np.tile(
            np.repeat(Wt2[e, 0][:, None], HALF, axis=1), (2, 1)
        )
    u = u.astype(ml_dtypes.bfloat16).reshape(E, 8, 16 * UF)

    bias = np.zeros((E, HALF, 4), dtype=np.float32)
    for e in range(E):
        bias[e, :, 0] = brgb[e]
        bias[e, :, 1] = btir[e]
        bias[e, 0:QUARTER, 2] = bt1[e]
        bias[e, QUARTER:HALF, 2] = bt1[e]
        bias[e, :, 3] = bt2[e, 0]
    bias = bias.reshape(E, 1, HALF * 4)

    wgt = Wg.T.astype(np.float32)                   # [256, 5]
    wg_p = np.ascontiguousarray(
        np.stack([wgt[:HALF], wgt[HALF:]], axis=1)
    ).astype(ml_dtypes.bfloat16)                    # [128, 2, 5]
    bgx = np.ascontiguousarray((bg * float(HW))[None, :].astype(np.float32))
    iota = (np.arange(E, dtype=np.float32) - 200.0)[None, :]

    xp = np.ascontiguousarray(x.reshape(B, C, HW)).astype(ml_dtypes.bfloat16)
    common = {"u": u, "bias": bias, "wg": wg_p, "bg": bgx, "iota": iota}
    in_maps = []
    for b in range(B):
        m = dict(common)
        m["x"] = xp[b]
        in_maps.append(m)
    return in_maps


_NC_CACHE = {}


def _get_nc():
    if "nc" not in _NC_CACHE:
        _NC_CACHE["nc"] = build_nc()
    return _NC_CACHE["nc"]


def kernel(x, Wg, bg, Wrgb, brgb, Wtir, btir, Wt1, bt1, Wt2, bt2, **run_kw):
    nc = _get_nc()
    in_maps = _pack_inputs(
        np.asarray(x), np.asarray(Wg), np.asarray(bg), np.asarray(Wrgb),
        np.asarray(brgb), np.asarray(Wtir), np.asarray(btir),
        np.asarray(Wt1), np.asarray(bt1), np.asarray(Wt2), np.asarray(bt2),
    )
    res = run_bass_kernel_spmd(nc, in_maps, core_ids=list(range(NCORES)), **run_kw)
    out = np.stack([np.asarray(r["out"]).astype(np.float32) for r in res.results], axis=0)
    if run_kw:
        kernel.last_results = res
    return out.reshape(B, HALF, H, W)


# revision 35
# speedup vs baseline: 1.0737x; 1.0737x over previous
"""MoE routing kernel for Trainium2 (8 NeuronCores, batch-parallel).

Problem: nn_MoE_47278999994656.
  x [8, 256, 80, 80] f32 + gate Linear(256->5) + 5 experts
  (residual conv1x1 on each 128-ch half, gated by a sigmoid transform),
  top-1 masked-softmax gate => weights are EXACTLY one-hot, so
  out[b] = expert_{argmax_e logits[b,e]}(x[b]).

Core i computes batch item i. Per core, three phases:
  A) x is host-cast to bf16 and streamed on the two HWDGE queues
     (sync: half0, scalar: half1) in 4 pieces each; the gate runs as
     PSUM-accumulated [5,512] matmuls per piece, with junk matmuls
     interleaved to keep the PE p-state ramped.
  B) argmax is computed as a register (mask -> iota+min -> int32 ->
     values_load); the selected expert's weights are then fetched from
     DRAM with ONE dynamic-index DMA per region (no 5-expert masked
     sum on the vector engines).
  C) expert pipeline, stage-skewed (D(i), H(i-1), A(i-2)) with
     elementwise work spread over Vector/Scalar/GpSimd; output is
     written bf16 in [128,1024] blocks alternating both HWDGE queues.
"""

import numpy as np

import concourse.bacc as bacc_mod
import concourse.bass as bass
import concourse.mybir as mybir
import concourse.tile as tile
from concourse.bass import ds
from concourse.bass_utils import run_bass_kernel_spmd

B, C, H, W = 8, 256, 80, 80
HW = H * W          # 6400
HALF = 128
QUARTER = 64
E = 5
NCORES = 8

# expert-layer chunks: 12 x 512 + 1 x 256 (psum bank holds 512 f32)
CHUNKS = [(i * 512, 512) for i in range(12)] + [(6144, 256)]
# input DMA: the two HWDGE queues run ~190 GB/s each with ~1.5k-col
# pieces; SWDGE starts ~10us late (engine preamble) so it only carries
# the tiny gate weights. sync: h0, scalar: h1, 4 pieces each.
HP = [(0, 1472), (1472, 1472), (2944, 1472), (4416, 1984)]
# (half, off, n) in arrival order for the gate matmuls
ARRIVAL = []
for _o, _n in HP:
    ARRIVAL.append((0, _o, _n))
    ARRIVAL.append((1, _o, _n))

# U_all free-dim layout (per expert, partition dim = 128):
#   [0:128)    (I + Wrgb)^T        [c, o]
#   [128:256)  (I + Wtir)^T        [c, o]
#   [256:320)  Wt1^T               [o, m]   (m = 64)
#   [320:448)  Wt2 replicated      [m, :]   rows 0:64 and 64:128 both = rep
UF = 448
U_WT1 = 256
U_WT2 = 320

F32 = mybir.dt.float32
BF16 = mybir.dt.bfloat16
I32 = mybir.dt.int32


def build_nc() -> bass.Bass:
    nc = bacc_mod.Bacc(num_devices=NCORES)

    x_d = nc.dram_tensor("x", [C, HW], BF16, kind="ExternalInput")
    u_d = nc.dram_tensor("u", [E, HALF, UF], BF16, kind="ExternalInput")
    bias_d = nc.dram_tensor("bias", [E, HALF, 4], F32, kind="ExternalInput")
    wg_d = nc.dram_tensor("wg", [HALF, 2, E], BF16, kind="ExternalInput")
    bg_d = nc.dram_tensor("bg", [1, E], F32, kind="ExternalInput")
    iota_d = nc.dram_tensor("iota", [1, E], F32, kind="ExternalInput")
    out_d = nc.dram_tensor("out", [HALF, HW], BF16, kind="ExternalOutput")

    with tile.TileContext(nc) as tc:
        with (
            tc.tile_pool(name="big", bufs=1) as big,
            tc.tile_pool(name="const", bufs=1) as const,
            tc.tile_pool(name="small", bufs=1) as small,
            tc.tile_pool(name="hpool", bufs=6) as hpool,
            tc.tile_pool(name="ppool", bufs=6) as ppool,
            tc.tile_pool(name="dps_p", bufs=4, space="PSUM") as dps_p,
            tc.tile_pool(name="aps_p", bufs=2, space="PSUM") as aps_p,
        ):
            # ---- persistent SBUF ----
            xb = big.tile([HALF, 2, HW], BF16)       # 25.6 KB/part
            dsb = big.tile([HALF, 2, HW], BF16)      # 25.6 KB/part
            osb = big.tile([HALF, HW], BF16)         # 12.8 KB/part
            usel = const.tile([HALF, UF], BF16)
            bsel = const.tile([HALF, 4], F32)
            wg = const.tile([HALF, 2, E], BF16)
            bgx = const.tile([1, E], F32)
            iot = const.tile([1, E], F32)

            # ---- phase A: x on both HWDGE queues, tiny weights on SWDGE ----
            nc.gpsimd.dma_start(out=wg[:], in_=wg_d[:])
            nc.gpsimd.dma_start(out=bgx[:], in_=bg_d[:])
            nc.gpsimd.dma_start(out=iot[:], in_=iota_d[:])
            # dummy activations so the act tables load during the DMA phase,
            # not at the first real activation inside the pipeline
            awarm = small.tile([1, 8], F32)
            nc.vector.memset(awarm, 0.0)
            nc.scalar.activation(
                out=awarm[0:1, 0:4], in_=awarm[0:1, 0:4],
                func=mybir.ActivationFunctionType.Identity,
            )
            nc.scalar.activation(
                out=awarm[0:1, 4:8], in_=awarm[0:1, 4:8],
                func=mybir.ActivationFunctionType.Sigmoid,
            )
            for o, n in HP:
                nc.sync.dma_start(
                    out=xb[:, 0, o : o + n], in_=x_d[0:HALF, o : o + n]
                )
                nc.scalar.dma_start(
                    out=xb[:, 1, o : o + n], in_=x_d[HALF:C, o : o + n]
                )

            # gate: yg[5, 512] += WgT_half^T @ xb sub-pieces (PSUM
            # accumulate), in arrival order. Junk matmuls keep the PE
            # continuously busy so its p-state ramps toward full clock
            # by the time the expert pipeline starts.
            yg = dps_p.tile([E, 512], F32, tag="dps", name="yg")
            ones1 = small.tile([1, 1], F32)
            nc.vector.memset(ones1, 1.0)

            def emit_junk(k, cols=512):
                for _ in range(k):
                    jt = dps_p.tile([E, 512], F32, tag="dps", name="junk")
                    nc.tensor.matmul(
                        jt[:, 0:cols], lhsT=wg[:, 0, :], rhs=xb[:, 0, 0:cols]
                    )

            # warm the PE before the first x piece lands (tiny wg junk)
            for _ in range(8):
                jt = dps_p.tile([E, 512], F32, tag="dps", name="jwarm")
                nc.tensor.matmul(jt[:, 0:E], lhsT=wg[:, 0, :],
                                 rhs=wg[:, 0, :])

            subs = []
            piece_of = []
            for pi, (h, o, n) in enumerate(ARRIVAL):
                done = 0
                while done < n:
                    m = min(512, n - done)
                    subs.append((h, o + done, m))
                    piece_of.append(pi)
                    done += m
            for k, (h, o, n) in enumerate(subs):
                nc.tensor.matmul(
                    yg[:, 0:n],
                    lhsT=wg[:, h, :],
                    rhs=xb[:, h, o : o + n],
                    start=(k == 0),
                    stop=False,
                )

            # fold the gate bias into column 0 of yg (saves a vector op)
            nc.tensor.matmul(
                yg[:, 0:1], lhsT=bgx[0:1, :], rhs=ones1[0:1, 0:1],
                start=False, stop=True,
            )

            # ---- phase B: argmax -> register -> dynamic weight fetch ----
            t32a = small.tile([32, 32], F32)
            t32b = small.tile([32, 32], F32)
            nc.vector.memset(t32a, 0.0)
            nc.vector.reduce_sum(t32a[0:E, 0:1], yg, axis=mybir.AxisListType.X)
            nc.vector.transpose(t32b, t32a)
            lmax = small.tile([1, 1], F32)
            nc.vector.reduce_max(lmax, t32b[0:1, 0:E], axis=mybir.AxisListType.X)
            # selrow[e] = is_eq(l[e], max) * (e - 200): flagged -> e-200 (<0)
            # reduce_min then picks the FIRST flagged index (ref tie-break).
            selrow = small.tile([1, E], F32)
            nc.vector.scalar_tensor_tensor(
                out=selrow, in0=t32b[0:1, 0:E], scalar=lmax[0:1, 0:1],
                in1=iot, op0=mybir.AluOpType.is_equal,
                op1=mybir.AluOpType.mult,
            )
            selv = small.tile([1, 1], F32)
            nc.vector.tensor_reduce(
                selv, selrow, axis=mybir.AxisListType.X, op=mybir.AluOpType.min
            )
            selint = small.tile([1, 1], I32)
            nc.vector.tensor_scalar(
                out=selint, in0=selv, scalar1=200.0, scalar2=None,
                op0=mybir.AluOpType.add,
            )
            sel = nc.values_load(
                selint[0:1, 0:1],
                engines=[mybir.EngineType.Pool, mybir.EngineType.SP,
                         mybir.EngineType.Activation],
                min_val=0, max_val=E - 1,
                skip_runtime_bounds_check=True,
            )
            # keep PE hot across the select window
            emit_junk(4)
            # selected expert's weights: dynamic-index fetch of the flat
            # block (few big descriptors), then static SBUF->SBUF reshapes
            # into lhsT layout. D columns land first so D(0) starts asap.
            nc.gpsimd.dma_start(out=bsel[:], in_=bias_d[ds(sel, 1)])
            nc.sync.dma_start(
                out=usel[:, 0:HALF], in_=u_d[ds(sel, 1), :, 0:HALF]
            )
            nc.scalar.dma_start(
                out=usel[:, HALF:256], in_=u_d[ds(sel, 1), :, HALF:256]
            )
            nc.gpsimd.dma_start(
                out=usel[:, 256:UF], in_=u_d[ds(sel, 1), :, 256:UF]
            )

            # ---- phase C: stage-skewed pipeline. PE iteration i issues
            # D(i), H(i-1), A(i-2). ----
            hsbl = [None] * len(CHUNKS)
            ssbl = [None] * len(CHUNKS)
            for i in range(len(CHUNKS) + 2):
                if i < len(CHUNKS):
                    off, n = CHUNKS[i]
                    dr = dps_p.tile([HALF, 512], F32, tag="dps", name="dr")
                    nc.tensor.matmul(
                        dr[:, 0:n], lhsT=usel[:, 0:HALF],
                        rhs=xb[:, 0, off : off + n]
                    )
                    dt = dps_p.tile([HALF, 512], F32, tag="dps", name="dt")
                    nc.tensor.matmul(
                        dt[:, 0:n], lhsT=usel[:, HALF : 2 * HALF],
                        rhs=xb[:, 1, off : off + n],
                    )
                    nc.vector.tensor_scalar_add(
                        dsb[:, 0, off : off + n], dr[:, 0:n], bsel[:, 0:1]
                    )
                    nc.scalar.activation(
                        out=dsb[:, 1, off : off + n], in_=dt[:, 0:n],
                        func=mybir.ActivationFunctionType.Identity,
                        bias=bsel[:, 1:2],
                    )
                if 0 <= i - 1 < len(CHUNKS):
                    ci = i - 1
                    off, n = CHUNKS[ci]
                    hps = dps_p.tile([HALF, 512], F32, tag="dps", name="hps")
                    nc.tensor.matmul(
                        hps[0:QUARTER, 0:n],
                        lhsT=usel[:, U_WT1 : U_WT1 + QUARTER],
                        rhs=dsb[:, 0, off : off + n],
                    )
                    nc.tensor.matmul(
                        hps[QUARTER:HALF, 0:n],
                        lhsT=usel[:, U_WT1 : U_WT1 + QUARTER],
                        rhs=dsb[:, 1, off : off + n],
                        tile_position=(0, QUARTER),
                    )
                    hsbl[ci] = hpool.tile([HALF, 512], BF16, tag="hsb", name="hsb")
                    if ci % 2 == 0:
                        nc.vector.tensor_scalar(
                            out=hsbl[ci][:, 0:n], in0=hps[:, 0:n],
                            scalar1=bsel[:, 2:3], scalar2=0.0,
                            op0=mybir.AluOpType.add, op1=mybir.AluOpType.max,
                        )
                    else:
                        nc.scalar.activation(
                            out=hsbl[ci][:, 0:n], in_=hps[:, 0:n],
                            func=mybir.ActivationFunctionType.Relu,
                            bias=bsel[:, 2:3],
                        )
                if 0 <= i - 2 < len(CHUNKS):
                    ci = i - 2
                    off, n = CHUNKS[ci]
                    aps = aps_p.tile([HALF, 2, 512], F32, tag="aps", name="aps")
                    nc.tensor.matmul(
                        aps[:, 0, 0:n],
                        lhsT=usel[0:QUARTER, U_WT2 : U_WT2 + HALF],
                        rhs=hsbl[ci][0:QUARTER, 0:n],
                        tile_position=(0, 0),
                    )
                    nc.tensor.matmul(
                        aps[:, 1, 0:n],
                        lhsT=usel[QUARTER:HALF, U_WT2 : U_WT2 + HALF],
                        rhs=hsbl[ci][QUARTER:HALF, 0:n],
                        tile_position=(QUARTER, 0),
                    )
                    ssbl[ci] = ppool.tile([HALF, 2, 512], BF16, tag="ssb", name="ssb")
                    nc.scalar.activation(
                        out=ssbl[ci][:, :, 0:n], in_=aps[:, :, 0:n],
                        func=mybir.ActivationFunctionType.Sigmoid,
                        bias=bsel[:, 3:4],
                    )
                    prt = ppool.tile([HALF, 2, 512], BF16, tag="prt", name="prt")
                    nc.vector.tensor_mul(
                        prt[:, :, 0:n], dsb[:, :, off : off + n],
                        ssbl[ci][:, :, 0:n]
                    )
                    nc.gpsimd.tensor_add(
                        osb[:, off : off + n], prt[:, 0, 0:n], prt[:, 1, 0:n]
                    )
                    if ci % 2 == 1:
                        bo = CHUNKS[ci - 1][0]
                        bn = off + n - bo
                        eng = nc.sync if (ci // 2) % 2 == 0 else nc.scalar
                        eng.dma_start(
                            out=out_d[:, bo : bo + bn], in_=osb[:, bo : bo + bn]
                        )
                    elif ci == len(CHUNKS) - 1:
                        nc.sync.dma_start(
                            out=out_d[:, off : off + n], in_=osb[:, off : off + n]
                        )

    nc.compile()
    return nc


def _pack_inputs(x, Wg, bg, Wrgb, brgb, Wtir, btir, Wt1, bt1, Wt2, bt2):
    import ml_dtypes
    eye = np.eye(HALF, dtype=np.float32)
    u = np.zeros((E, HALF, UF), dtype=np.float32)
    for e in range(E):
        u[e, :, 0:HALF] = Wrgb[e].T + eye
        u[e, :, HALF : 2 * HALF] = Wtir[e].T + eye
        u[e, :, U_WT1 : U_WT1 + QUARTER] = Wt1[e].T
        u[e, :, U_WT2 : U_WT2 + HALF] = np.tile(
            np.repeat(Wt2[e, 0][:, None], HALF, axis=1), (2, 1)
        )
    u = u.astype(ml_dtypes.bfloat16)

    bias = np.zeros((E, HALF, 4), dtype=np.float32)
    for e in range(E):
        bias[e, :, 0] = brgb[e]
        bias[e, :, 1] = btir[e]
        bias[e, 0:QUARTER, 2] = bt1[e]
        bias[e, QUARTER:HALF, 2] = bt1[e]
        bias[e, :, 3] = bt2[e, 0]

    wgt = Wg.T.astype(np.float32)                   # [256, 5]
    wg_p = np.ascontiguousarray(
        np.stack([wgt[:HALF], wgt[HALF:]], axis=1)
    ).astype(ml_dtypes.bfloat16)                    # [128, 2, 5]
    bgx = np.ascontiguousarray((bg * float(HW))[None, :].astype(np.float32))
    iota = (np.arange(E, dtype=np.float32) - 200.0)[None, :]

    xp = np.ascontiguousarray(x.reshape(B, C, HW)).astype(ml_dtypes.bfloat16)
    common = {"u": u, "bias": bias, "wg": wg_p, "bg": bgx, "iota": iota}
    in_maps = []
    for b in range(B):
        m = dict(common)
        m["x"] = xp[b]
        in_maps.append(m)
    return in_maps


_NC_CACHE = {}


def _get_nc():
    if "nc" not in _NC_CACHE:
        _NC_CACHE["nc"] = build_nc()
    return _NC_CACHE["nc"]


def kernel(x, Wg, bg, Wrgb, brgb, Wtir, btir, Wt1, bt1, Wt2, bt2, **run_kw):
    nc = _get_nc()
    in_maps = _pack_inputs(
        np.asarray(x), np.asarray(Wg), np.asarray(bg), np.asarray(Wrgb),
        np.asarray(brgb), np.asarray(Wtir), np.asarray(btir),
        np.asarray(Wt1), np.asarray(bt1), np.asarray(Wt2), np.asarray(bt2),
    )
    res = run_bass_kernel_spmd(nc, in_maps, core_ids=list(range(NCORES)), **run_kw)
    out = np.stack([np.asarray(r["out"]).astype(np.float32) for r in res.results], axis=0)
    if run_kw:
        kernel.last_results = res
    return out.reshape(B, HALF, H, W)
